# revision 6
# baseline (speedup 1.0000x reference)
# Trainium2 Bass kernel: single-head causal self-attention (nanoGPT Head).
#
#   x: [8, 4096, 64], Wq/Wk/Wv: [64, 128] -> out: [8, 4096, 128]
#
# Sharding: data-parallel, one batch element per NeuronCore (8 cores).
# Per core (T=4096, C=64, H=128):
#   setup:  xT = x.T (PE transposes), qT/kT = W.T @ xT (f32r, full rate),
#           v = xT.T @ Wv (cast to fp16)
#   flash loop over 32 query tiles (128 queries each), causal:
#     S[q,k] chunk = qT_tile.T @ kT_chunk     (f32r, N<=512, PSUM)
#     diag mask: add -1e9 upper triangle
#     P = exp(S*scale) -> fp16 SBUF, ACT accumulates row sums l
#     P.T via xbar DMA transpose (fp16)
#     O += P.T.T @ v_tile  (fp16 matmuls accumulating in PSUM)
#     out_tile = O * (1/l)  (per-partition scalar on DVE)
# Softmax max-subtraction is skipped: scores ~ N(0,1) (|s|<~7), fp32 exp is
# safe, and exp(s)/sum(exp(s)) is mathematically identical.
#
# Host<->device I/O over the axon tunnel (~35 MB/s, high variance) is the
# wall-clock bottleneck, so:
#   - x and W are shipped as fp16 and upcast on-chip; out is written as fp16
#     and upcast to f32 on the host (fp16 over bf16: same bytes, 4x finer
#     mantissa for N(0,1)-scale data);
#   - the jitted callables are built ONCE and cached, so repeat calls skip
#     retrace/recompile/NEFF-reload;
#   - no zero output-donation buffers are shipped (the kernel writes every
#     output element, so uninitialized result buffers are fine);
#   - each core runs its own single-device jit, dispatched from 8 threads, so
#     uploads, executes, and downloads of different cores overlap in the
#     tunnel instead of serializing.

import sys
import numpy as np
from concurrent.futures import ThreadPoolExecutor
from contextlib import ExitStack

for _p in ("/opt/trn_rl_repo",):
    if _p not in sys.path:
        sys.path.append(_p)

B, T, C, H = 8, 4096, 64, 128
NT = T // 128  # 32 query/key tiles
SCALE = float(H) ** -0.5
N_CORES = 8

_cache = {}


def _build():
    import concourse.bass as bass  # noqa: F401
    import concourse.mybir as mybir
    import concourse.tile as tile
    from concourse import bacc
    from concourse.masks import make_identity, make_causal_mask

    f32 = mybir.dt.float32
    f32r = mybir.dt.float32r
    fp16 = mybir.dt.float16
    EXP = mybir.ActivationFunctionType.Exp
    AXX = mybir.AxisListType.X

    nc = bacc.Bacc("TRN2", target_bir_lowering=False)
    x_d = nc.dram_tensor("xb", [T, C], fp16, kind="ExternalInput")
    wq_d = nc.dram_tensor("Wq", [C, H], fp16, kind="ExternalInput")
    wk_d = nc.dram_tensor("Wk", [C, H], fp16, kind="ExternalInput")
    wv_d = nc.dram_tensor("Wv", [C, H], fp16, kind="ExternalInput")
    out_d = nc.dram_tensor("out", [T, H], fp16, kind="ExternalOutput")

    with ExitStack() as ctx:
        tc = ctx.enter_context(tile.TileContext(nc))
        const = ctx.enter_context(tc.tile_pool(name="const", bufs=1))
        big = ctx.enter_context(tc.tile_pool(name="big", bufs=1))

        wq_sb = const.tile([C, H], fp16, tag="wq")
        wk_sb = const.tile([C, H], fp16, tag="wk")
        wv_sb = const.tile([C, H], fp16, tag="wv")
        nc.sync.dma_start(out=wq_sb, in_=wq_d[:, :])
        nc.sync.dma_start(out=wk_sb, in_=wk_d[:, :])
        nc.sync.dma_start(out=wv_sb, in_=wv_d[:, :])
        wq_r = const.tile([C, H], f32r, tag="wq_r")
        wk_r = const.tile([C, H], f32r, tag="wk_r")
        wv_r = const.tile([C, H], f32r, tag="wv_r")
        nc.vector.tensor_copy(out=wq_r, in_=wq_sb)
        nc.vector.tensor_copy(out=wk_r, in_=wk_sb)
        nc.vector.tensor_copy(out=wv_r, in_=wv_sb)
        ident = const.tile([128, 128], f32, tag="ident")
        make_identity(nc, ident)
        maskneg = const.tile([128, 128], f32, tag="maskneg")
        make_causal_mask(nc, maskneg, mask_val=-1e9)

        qT = big.tile([128, T], f32r, tag="qT")
        kT = big.tile([128, T], f32r, tag="kT")
        v_sb = big.tile([128, NT, H], fp16, tag="v_sb")
        out_acc = big.tile([128, NT, H], fp16, tag="out_acc")

        # ---- setup: transpose x, project q/k/v ----
        with ExitStack() as sctx:
            xt_pool = sctx.enter_context(tc.tile_pool(name="xt_pool", bufs=1))
            setup_ps = sctx.enter_context(
                tc.tile_pool(name="setup_ps", bufs=2, space="PSUM")
            )
            x_hf = xt_pool.tile([128, NT, C], fp16, tag="x_hf")
            nc.sync.dma_start(
                out=x_hf, in_=x_d[:, :].rearrange("(n p) c -> p n c", p=128)
            )
            x_sb = xt_pool.tile([128, NT, C], f32, tag="x_sb")
            nc.vector.tensor_copy(out=x_sb, in_=x_hf)
            xT = xt_pool.tile([C, T], f32r, tag="xT")
            for i in range(NT):
                ps_t = setup_ps.tile([C, 128], f32, tag="ps_t")
                nc.tensor.transpose(ps_t, x_sb[:, i, :], ident)
                nc.vector.tensor_copy(out=xT[:, i * 128 : (i + 1) * 128], in_=ps_t)
            for c8 in range(T // 512):
                sl = slice(c8 * 512, (c8 + 1) * 512)
                ps_q = setup_ps.tile([128, 512], f32, tag="ps_q")
                nc.tensor.matmul(
                    ps_q,
                    lhsT=wq_r,
                    rhs=xT[:, sl],
                    start=True,
                    stop=True,
                )
                nc.vector.tensor_copy(out=qT[:, sl], in_=ps_q)
                ps_k = setup_ps.tile([128, 512], f32, tag="ps_k")
                nc.tensor.matmul(
                    ps_k,
                    lhsT=wk_r,
                    rhs=xT[:, sl],
                    start=True,
                    stop=True,
                )
                nc.vector.tensor_copy(out=kT[:, sl], in_=ps_k)
            for i in range(NT):
                ps_v = setup_ps.tile([128, H], f32, tag="ps_v")
                nc.tensor.matmul(
                    ps_v,
                    lhsT=xT[:, i * 128 : (i + 1) * 128],
                    rhs=wv_r,
                    start=True,
                    stop=True,
                )
                nc.vector.tensor_copy(out=v_sb[:, i, :], in_=ps_v)

        # ---- flash loop over query tiles ----
        ps_s_pool = ctx.enter_context(tc.tile_pool(name="ps_s", bufs=3, space="PSUM"))
        ps_o_pool = ctx.enter_context(tc.tile_pool(name="ps_o", bufs=2, space="PSUM"))
        p_pool = ctx.enter_context(tc.tile_pool(name="p_pool", bufs=3))
        pt_pool = ctx.enter_context(tc.tile_pool(name="pt_pool", bufs=3))
        lil = ctx.enter_context(tc.tile_pool(name="lil", bufs=2))

        for i in range(NT):
            nk = i + 1  # causal: key tiles 0..i
            nchunks = (nk + 3) // 4
            ps_o = ps_o_pool.tile([128, H], f32, tag="ps_o")
            l_parts = lil.tile([128, 8], f32, tag="l_parts")
            for c in range(nchunks):
                k0 = c * 512
                ck = min(512, nk * 128 - k0)
                ntile = ck // 128
                ps_s = ps_s_pool.tile([128, 512], f32, tag="ps_s")
                nc.tensor.matmul(
                    ps_s[:, :ck],
                    lhsT=qT[:, i * 128 : (i + 1) * 128],
                    rhs=kT[:, k0 : k0 + ck],
                    start=True,
                    stop=True,
                )
                if c == nchunks - 1:
                    nc.vector.tensor_add(
                        out=ps_s[:, ck - 128 : ck],
                        in0=ps_s[:, ck - 128 : ck],
                        in1=maskneg,
                    )
                p_sb = p_pool.tile([128, 512], fp16, tag="p_sb")
                nc.scalar.activation(
                    out=p_sb[:, :ck],
                    in_=ps_s[:, :ck],
                    func=EXP,
                    scale=SCALE,
                    accum_out=l_parts[:, c : c + 1],
                )
                pt = pt_pool.tile([128, 4, 128], fp16, tag="pt")
                nc.sync.dma_start(
                    out=pt[:, :ntile, :], in_=p_sb[:, :ck], transpose=True
                )
                for jj in range(ntile):
                    j = c * 4 + jj
                    nc.tensor.matmul(
                        ps_o,
                        lhsT=pt[:, jj, :],
                        rhs=v_sb[:, j, :],
                        start=(j == 0),
                        stop=(j == i),
                    )
            recip = lil.tile([128, 1], f32, tag="recip")
            if nchunks > 1:
                l_sum = lil.tile([128, 1], f32, tag="l_sum")
                nc.vector.reduce_sum(out=l_sum, in_=l_parts[:, :nchunks], axis=AXX)
                nc.vector.reciprocal(recip, l_sum)
            else:
                nc.vector.reciprocal(recip, l_parts[:, 0:1])
            nc.vector.tensor_scalar_mul(out_acc[:, i, :], ps_o, recip)

        nc.sync.dma_start(
            out=out_d[:, :].rearrange("(n p) h -> p n h", p=128), in_=out_acc
        )
    nc.finalize()
    return nc


def _get_nc():
    if "nc" not in _cache:
        _cache["nc"] = _build()
    return _cache["nc"]


def _get_callable():
    """Build the jitted per-core callables once; reuse across calls.

    Each core gets its own single-device shard_map jit (a plain
    single-device jax.jit of the bass_exec body crashes the exec unit on
    the axon terminal; the shard_map-lowered form runs fine)."""
    if "call" in _cache:
        return _cache["call"]

    import jax
    from jax.sharding import Mesh, PartitionSpec
    from jax.experimental.shard_map import shard_map
    import concourse.mybir as mybir
    from concourse.bass2jax import (
        _bass_exec_p,
        install_neuronx_cc_hook,
        partition_id_tensor,
    )

    install_neuronx_cc_hook()
    nc = _get_nc()
    partition_name = nc.partition_id_tensor.name if nc.partition_id_tensor else None

    in_names = []
    out_names = []
    out_avals = []
    for alloc in nc.m.functions[0].allocations:
        if not isinstance(alloc, mybir.MemoryLocationSet):
            continue
        name = alloc.memorylocations[0].name
        if alloc.kind == "ExternalInput":
            if name != partition_name:
                in_names.append(name)
        elif alloc.kind == "ExternalOutput":
            out_names.append(name)
            out_avals.append(
                jax.core.ShapedArray(tuple(alloc.tensor_shape), mybir.dt.np(alloc.dtype))
            )
    all_in_names = list(in_names)
    if partition_name is not None:
        all_in_names.append(partition_name)

    def _body(*args):
        operands = list(args)
        if partition_name is not None:
            operands.append(partition_id_tensor())
        outs = _bass_exec_p.bind(
            *operands,
            out_avals=tuple(out_avals),
            in_names=tuple(all_in_names),
            out_names=tuple(out_names),
            lowering_input_output_aliases=(),
            sim_require_finite=True,
            sim_require_nnan=True,
            nc=nc,
        )
        return tuple(outs)

    devices = jax.devices()[:N_CORES]
    assert len(devices) == N_CORES, f"need {N_CORES} devices, got {len(devices)}"
    calls = []
    for dev in devices:
        mesh = Mesh(np.asarray([dev]), ("core",))
        calls.append(
            jax.jit(
                shard_map(
                    _body,
                    mesh=mesh,
                    in_specs=(PartitionSpec("core"),) * len(in_names),
                    out_specs=(PartitionSpec("core"),) * len(out_names),
                    check_rep=False,
                ),
                keep_unused=True,
            )
        )
    pool = ThreadPoolExecutor(max_workers=N_CORES)
    _cache["call"] = (calls, in_names, pool)
    return _cache["call"]


def _run(inputs, trace=False):
    if trace:
        return _run_traced(inputs)
    calls, in_names, pool = _get_callable()

    x = np.asarray(inputs["x"], dtype=np.float32).astype(np.float16)
    w16 = {
        k: np.asarray(inputs[k], dtype=np.float32).astype(np.float16)
        for k in ("Wq", "Wk", "Wv")
    }
    out = np.empty((N_CORES, T, H), dtype=np.float32)

    def one(core):
        arrs = {"xb": np.ascontiguousarray(x[core]), **w16}
        o = calls[core](*[arrs[n] for n in in_names])
        out[core] = np.asarray(o[0]).astype(np.float32)

    futs = [pool.submit(one, b) for b in range(N_CORES)]
    for f in futs:
        f.result()

    class _Res:
        exec_time_ns = None
        results = None

    return out, _Res()


def _run_traced(inputs):
    """Profiled path via run_bass_kernel_spmd (NTFF trace)."""
    from concourse.bass_utils import run_bass_kernel_spmd

    x = np.asarray(inputs["x"], dtype=np.float32).astype(np.float16)
    wq = np.asarray(inputs["Wq"], dtype=np.float32).astype(np.float16)
    wk = np.asarray(inputs["Wk"], dtype=np.float32).astype(np.float16)
    wv = np.asarray(inputs["Wv"], dtype=np.float32).astype(np.float16)
    in_maps = [
        {"xb": np.ascontiguousarray(x[b]), "Wq": wq, "Wk": wk, "Wv": wv}
        for b in range(N_CORES)
    ]
    res = run_bass_kernel_spmd(
        _get_nc(), in_maps, core_ids=list(range(N_CORES)), trace=True
    )
    out = np.stack([r["out"] for r in res.results], axis=0).astype(np.float32)
    return out, res


def kernel(x, Wq, Wk, Wv):
    out, _ = _run({"x": x, "Wq": Wq, "Wk": Wk, "Wv": Wv})
    return out


# revision 7
# speedup vs baseline: 1.1864x; 1.1864x over previous
# Trainium2 Bass kernel: single-head causal self-attention (nanoGPT Head).
#
#   x: [8, 4096, 64], Wq/Wk/Wv: [64, 128] -> out: [8, 4096, 128]
#
# Sharding: data-parallel, one batch element per NeuronCore (8 cores).
# Per core (T=4096, C=64, H=128):
#   setup:  xT = x.T (PE transposes), qT/kT = W.T @ xT (f32r, full rate),
#           v = xT.T @ Wv (cast to fp16)
#   flash loop over 32 query tiles (128 queries each), causal:
#     S[q,k] chunk = qT_tile.T @ kT_chunk     (f32r, N<=512, PSUM)
#     diag mask: add -1e9 upper triangle
#     P = exp(S*scale) -> fp16 SBUF, ACT accumulates row sums l
#     P.T via xbar DMA transpose (fp16)
#     O += P.T.T @ v_tile  (fp16 matmuls accumulating in PSUM)
#     out_tile = O * (1/l)  (per-partition scalar on DVE)
# Softmax max-subtraction is skipped: scores ~ N(0,1) (|s|<~7), fp32 exp is
# safe, and exp(s)/sum(exp(s)) is mathematically identical.
#
# Host<->device I/O over the axon tunnel (~35 MB/s, high variance) is the
# wall-clock bottleneck, so:
#   - x and W are shipped as fp16 and upcast on-chip; out is written as fp16
#     and upcast to f32 on the host (fp16 over bf16: same bytes, 4x finer
#     mantissa for N(0,1)-scale data);
#   - the jitted callables are built ONCE and cached, so repeat calls skip
#     retrace/recompile/NEFF-reload;
#   - no zero output-donation buffers are shipped (the kernel writes every
#     output element, so uninitialized result buffers are fine);
#   - each core runs its own single-device jit, dispatched from 8 threads, so
#     uploads, executes, and downloads of different cores overlap in the
#     tunnel instead of serializing.

import sys
import numpy as np
from concurrent.futures import ThreadPoolExecutor
from contextlib import ExitStack

for _p in ("/opt/trn_rl_repo",):
    if _p not in sys.path:
        sys.path.append(_p)

B, T, C, H = 8, 4096, 64, 128
NT = T // 128  # 32 query/key tiles
SCALE = float(H) ** -0.5
N_CORES = 8

_cache = {}


def _build():
    import concourse.bass as bass  # noqa: F401
    import concourse.mybir as mybir
    import concourse.tile as tile
    from concourse import bacc
    from concourse.masks import make_identity, make_causal_mask

    f32 = mybir.dt.float32
    f32r = mybir.dt.float32r
    fp16 = mybir.dt.float16
    EXP = mybir.ActivationFunctionType.Exp
    AXX = mybir.AxisListType.X

    nc = bacc.Bacc("TRN2", target_bir_lowering=False)
    x_d = nc.dram_tensor("xb", [T, C], fp16, kind="ExternalInput")
    wq_d = nc.dram_tensor("Wq", [C, H], fp16, kind="ExternalInput")
    wk_d = nc.dram_tensor("Wk", [C, H], fp16, kind="ExternalInput")
    wv_d = nc.dram_tensor("Wv", [C, H], fp16, kind="ExternalInput")
    out_d = nc.dram_tensor("out", [T, H], fp16, kind="ExternalOutput")

    with ExitStack() as ctx:
        tc = ctx.enter_context(tile.TileContext(nc))
        const = ctx.enter_context(tc.tile_pool(name="const", bufs=1))
        big = ctx.enter_context(tc.tile_pool(name="big", bufs=1))

        wq_sb = const.tile([C, H], fp16, tag="wq")
        wk_sb = const.tile([C, H], fp16, tag="wk")
        wv_sb = const.tile([C, H], fp16, tag="wv")
        nc.sync.dma_start(out=wq_sb, in_=wq_d[:, :])
        nc.sync.dma_start(out=wk_sb, in_=wk_d[:, :])
        nc.sync.dma_start(out=wv_sb, in_=wv_d[:, :])
        wq_r = const.tile([C, H], f32r, tag="wq_r")
        wk_r = const.tile([C, H], f32r, tag="wk_r")
        wv_r = const.tile([C, H], f32r, tag="wv_r")
        nc.vector.tensor_copy(out=wq_r, in_=wq_sb)
        nc.vector.tensor_copy(out=wk_r, in_=wk_sb)
        nc.vector.tensor_copy(out=wv_r, in_=wv_sb)
        ident = const.tile([128, 128], f32, tag="ident")
        make_identity(nc, ident)
        maskneg = const.tile([128, 128], f32, tag="maskneg")
        make_causal_mask(nc, maskneg, mask_val=-1e9)

        qT = big.tile([128, T], f32r, tag="qT")
        kT = big.tile([128, T], f32r, tag="kT")
        v_sb = big.tile([128, NT, H], fp16, tag="v_sb")
        out_acc = big.tile([128, NT, H], fp16, tag="out_acc")

        # ---- setup: transpose x, project q/k/v ----
        with ExitStack() as sctx:
            xt_pool = sctx.enter_context(tc.tile_pool(name="xt_pool", bufs=1))
            setup_ps = sctx.enter_context(
                tc.tile_pool(name="setup_ps", bufs=2, space="PSUM")
            )
            x_hf = xt_pool.tile([128, NT, C], fp16, tag="x_hf")
            nc.sync.dma_start(
                out=x_hf, in_=x_d[:, :].rearrange("(n p) c -> p n c", p=128)
            )
            x_sb = xt_pool.tile([128, NT, C], f32, tag="x_sb")
            nc.vector.tensor_copy(out=x_sb, in_=x_hf)
            xT = xt_pool.tile([C, T], f32r, tag="xT")
            for i in range(NT):
                ps_t = setup_ps.tile([C, 128], f32, tag="ps_t")
                nc.tensor.transpose(ps_t, x_sb[:, i, :], ident)
                nc.vector.tensor_copy(out=xT[:, i * 128 : (i + 1) * 128], in_=ps_t)
            for c8 in range(T // 512):
                sl = slice(c8 * 512, (c8 + 1) * 512)
                ps_q = setup_ps.tile([128, 512], f32, tag="ps_q")
                nc.tensor.matmul(
                    ps_q,
                    lhsT=wq_r,
                    rhs=xT[:, sl],
                    start=True,
                    stop=True,
                )
                nc.vector.tensor_copy(out=qT[:, sl], in_=ps_q)
                ps_k = setup_ps.tile([128, 512], f32, tag="ps_k")
                nc.tensor.matmul(
                    ps_k,
                    lhsT=wk_r,
                    rhs=xT[:, sl],
                    start=True,
                    stop=True,
                )
                nc.vector.tensor_copy(out=kT[:, sl], in_=ps_k)
            for i in range(NT):
                ps_v = setup_ps.tile([128, H], f32, tag="ps_v")
                nc.tensor.matmul(
                    ps_v,
                    lhsT=xT[:, i * 128 : (i + 1) * 128],
                    rhs=wv_r,
                    start=True,
                    stop=True,
                )
                nc.vector.tensor_copy(out=v_sb[:, i, :], in_=ps_v)

        # ---- flash loop over query tiles ----
        ps_s_pool = ctx.enter_context(tc.tile_pool(name="ps_s", bufs=3, space="PSUM"))
        ps_o_pool = ctx.enter_context(tc.tile_pool(name="ps_o", bufs=2, space="PSUM"))
        p_pool = ctx.enter_context(tc.tile_pool(name="p_pool", bufs=3))
        pt_pool = ctx.enter_context(tc.tile_pool(name="pt_pool", bufs=3))
        lil = ctx.enter_context(tc.tile_pool(name="lil", bufs=2))

        for i in range(NT):
            nk = i + 1  # causal: key tiles 0..i
            nchunks = (nk + 3) // 4
            ps_o = ps_o_pool.tile([128, H], f32, tag="ps_o")
            l_parts = lil.tile([128, 8], f32, tag="l_parts")
            for c in range(nchunks):
                k0 = c * 512
                ck = min(512, nk * 128 - k0)
                ntile = ck // 128
                ps_s = ps_s_pool.tile([128, 512], f32, tag="ps_s")
                nc.tensor.matmul(
                    ps_s[:, :ck],
                    lhsT=qT[:, i * 128 : (i + 1) * 128],
                    rhs=kT[:, k0 : k0 + ck],
                    start=True,
                    stop=True,
                )
                if c == nchunks - 1:
                    nc.vector.tensor_add(
                        out=ps_s[:, ck - 128 : ck],
                        in0=ps_s[:, ck - 128 : ck],
                        in1=maskneg,
                    )
                p_sb = p_pool.tile([128, 512], fp16, tag="p_sb")
                nc.scalar.activation(
                    out=p_sb[:, :ck],
                    in_=ps_s[:, :ck],
                    func=EXP,
                    scale=SCALE,
                    accum_out=l_parts[:, c : c + 1],
                )
                pt = pt_pool.tile([128, 4, 128], fp16, tag="pt")
                nc.sync.dma_start(
                    out=pt[:, :ntile, :], in_=p_sb[:, :ck], transpose=True
                )
                for jj in range(ntile):
                    j = c * 4 + jj
                    nc.tensor.matmul(
                        ps_o,
                        lhsT=pt[:, jj, :],
                        rhs=v_sb[:, j, :],
                        start=(j == 0),
                        stop=(j == i),
                    )
            recip = lil.tile([128, 1], f32, tag="recip")
            if nchunks > 1:
                l_sum = lil.tile([128, 1], f32, tag="l_sum")
                nc.vector.reduce_sum(out=l_sum, in_=l_parts[:, :nchunks], axis=AXX)
                nc.vector.reciprocal(recip, l_sum)
            else:
                nc.vector.reciprocal(recip, l_parts[:, 0:1])
            nc.vector.tensor_scalar_mul(out_acc[:, i, :], ps_o, recip)

        nc.sync.dma_start(
            out=out_d[:, :].rearrange("(n p) h -> p n h", p=128), in_=out_acc
        )
    nc.finalize()
    return nc


def _get_nc():
    if "nc" not in _cache:
        _cache["nc"] = _build()
    return _cache["nc"]


def _get_callable():
    """Build the jitted per-core callables once; reuse across calls.

    Each core gets its own single-device shard_map jit (a plain
    single-device jax.jit of the bass_exec body crashes the exec unit on
    the axon terminal; the shard_map-lowered form runs fine)."""
    if "call" in _cache:
        return _cache["call"]

    import jax
    from jax.sharding import Mesh, PartitionSpec
    from jax.experimental.shard_map import shard_map
    import concourse.mybir as mybir
    from concourse.bass2jax import (
        _bass_exec_p,
        install_neuronx_cc_hook,
        partition_id_tensor,
    )

    install_neuronx_cc_hook()
    nc = _get_nc()
    partition_name = nc.partition_id_tensor.name if nc.partition_id_tensor else None

    in_names = []
    out_names = []
    out_avals = []
    for alloc in nc.m.functions[0].allocations:
        if not isinstance(alloc, mybir.MemoryLocationSet):
            continue
        name = alloc.memorylocations[0].name
        if alloc.kind == "ExternalInput":
            if name != partition_name:
                in_names.append(name)
        elif alloc.kind == "ExternalOutput":
            out_names.append(name)
            out_avals.append(
                jax.core.ShapedArray(tuple(alloc.tensor_shape), mybir.dt.np(alloc.dtype))
            )
    all_in_names = list(in_names)
    if partition_name is not None:
        all_in_names.append(partition_name)

    def _body(*args):
        operands = list(args)
        if partition_name is not None:
            operands.append(partition_id_tensor())
        outs = _bass_exec_p.bind(
            *operands,
            out_avals=tuple(out_avals),
            in_names=tuple(all_in_names),
            out_names=tuple(out_names),
            lowering_input_output_aliases=(),
            sim_require_finite=True,
            sim_require_nnan=True,
            nc=nc,
        )
        return tuple(outs)

    devices = jax.devices()[:N_CORES]
    assert len(devices) == N_CORES, f"need {N_CORES} devices, got {len(devices)}"
    calls = []
    for dev in devices:
        mesh = Mesh(np.asarray([dev]), ("core",))
        calls.append(
            jax.jit(
                shard_map(
                    _body,
                    mesh=mesh,
                    in_specs=(PartitionSpec("core"),) * len(in_names),
                    out_specs=(PartitionSpec("core"),) * len(out_names),
                    check_rep=False,
                ),
                keep_unused=True,
            )
        )
    pool = ThreadPoolExecutor(max_workers=N_CORES)
    _cache["call"] = (calls, in_names, pool)
    return _cache["call"]


def _run(inputs, trace=False):
    if trace:
        return _run_traced(inputs)
    calls, in_names, pool = _get_callable()

    x = np.asarray(inputs["x"], dtype=np.float32).astype(np.float16)
    w16 = {
        k: np.asarray(inputs[k], dtype=np.float32).astype(np.float16)
        for k in ("Wq", "Wk", "Wv")
    }
    out = np.empty((N_CORES, T, H), dtype=np.float32)

    def one(core):
        arrs = {"xb": np.ascontiguousarray(x[core]), **w16}
        try:
            o = calls[core](*[arrs[n] for n in in_names])
            res = np.asarray(o[0])
        except Exception:
            o = calls[core](*[arrs[n] for n in in_names])
            res = np.asarray(o[0])
        out[core] = res.astype(np.float32)

    if "warm" not in _cache:
        # First call in this process: compile core 0's executable alone so
        # the NEFF lands in the on-disk cache before the other 7 compile.
        one(0)
        rest = [pool.submit(one, b) for b in range(1, N_CORES)]
        for f in rest:
            f.result()
        _cache["warm"] = True
    else:
        futs = [pool.submit(one, b) for b in range(N_CORES)]
        for f in futs:
            f.result()

    class _Res:
        exec_time_ns = None
        results = None

    return out, _Res()


def _run_traced(inputs):
    """Profiled path via run_bass_kernel_spmd (NTFF trace)."""
    from concourse.bass_utils import run_bass_kernel_spmd

    x = np.asarray(inputs["x"], dtype=np.float32).astype(np.float16)
    wq = np.asarray(inputs["Wq"], dtype=np.float32).astype(np.float16)
    wk = np.asarray(inputs["Wk"], dtype=np.float32).astype(np.float16)
    wv = np.asarray(inputs["Wv"], dtype=np.float32).astype(np.float16)
    in_maps = [
        {"xb": np.ascontiguousarray(x[b]), "Wq": wq, "Wk": wk, "Wv": wv}
        for b in range(N_CORES)
    ]
    res = run_bass_kernel_spmd(
        _get_nc(), in_maps, core_ids=list(range(N_CORES)), trace=True
    )
    out = np.stack([r["out"] for r in res.results], axis=0).astype(np.float32)
    return out, res


def kernel(x, Wq, Wk, Wv):
    out, _ = _run({"x": x, "Wq": Wq, "Wk": Wk, "Wv": Wv})
    return out


# revision 8
# speedup vs baseline: 1.3813x; 1.1643x over previous
# Trainium2 Bass kernel: single-head causal self-attention (nanoGPT Head).
#
#   x: [8, 4096, 64], Wq/Wk/Wv: [64, 128] -> out: [8, 4096, 128]
#
# Algebraic restructuring (exact): with M := Wq @ Wk^T * H^-0.5 ([64, 64]),
#   scores = (x@Wq) @ (x@Wk)^T * scale = x @ M @ x^T
#   out    = softmax(scores) @ (x@Wv) = (softmax(scores) @ x) @ Wv
# so the device consumes only x [T,64] and M [64,64], and returns
# z := softmax(scores) @ x of shape [T,64]; the host applies the thin
# epilogue out = z @ Wv ([T,64]@[64,128] sgemm, ~2 ms/core). This halves
# both device matmul phases AND halves the host<->device traffic (the axon
# tunnel at ~10-60 MB/s is the wall-clock bottleneck, not compute).
#
# Sharding: data-parallel, one batch element per NeuronCore (8 cores).
# Per core (T=4096, C=64):
#   setup:  xT = x.T (PE transposes), gT = M^T @ xT  (f32r, 64-contraction)
#   flash loop over 32 query tiles (128 queries each), causal:
#     S[q,k] chunk = gT_tile.T @ xT_chunk     (f32r, N<=512, PSUM)
#     diag mask: add -1e9 upper triangle
#     P = exp(S) -> fp16 SBUF, ACT accumulates row sums l
#     P.T via xbar DMA transpose (fp16)
#     Z += P.T.T @ x_tile  (fp16 matmuls accumulating in PSUM, width 64)
#     z_tile = Z * (1/l)  (per-partition scalar on DVE, fp16 out)
# Softmax max-subtraction is skipped: scores ~ N(0,1) (|s|<~7), fp32 exp is
# safe, and exp(s)/sum(exp(s)) is mathematically identical.
#
# Host<->device I/O engineering (wall time = transfers, not compute):
#   - x ships as fp16, z returns as fp16 (fp16 over bf16: same bytes, 4x
#     finer mantissa for N(0,1)-scale data); M ships as f32 (16 KB);
#   - the jitted callables are built ONCE and cached, so repeat calls skip
#     retrace/recompile/NEFF-reload;
#   - no zero output-donation buffers are shipped (the kernel writes every
#     output element, so uninitialized result buffers are fine);
#   - each core runs its own single-device shard_map jit (a plain
#     single-device jax.jit of the bass_exec body crashes the axon
#     terminal), dispatched from 8 threads so uploads, executes, and
#     downloads of different cores overlap in the tunnel.

import sys
import numpy as np
from concurrent.futures import ThreadPoolExecutor
from contextlib import ExitStack

for _p in ("/opt/trn_rl_repo",):
    if _p not in sys.path:
        sys.path.append(_p)

B, T, C, H = 8, 4096, 64, 128
NT = T // 128  # 32 query/key tiles
SCALE = float(H) ** -0.5
N_CORES = 8

_cache = {}


def _build():
    import concourse.bass as bass  # noqa: F401
    import concourse.mybir as mybir
    import concourse.tile as tile
    from concourse import bacc
    from concourse.masks import make_identity, make_causal_mask

    f32 = mybir.dt.float32
    f32r = mybir.dt.float32r
    fp16 = mybir.dt.float16
    EXP = mybir.ActivationFunctionType.Exp
    AXX = mybir.AxisListType.X

    nc = bacc.Bacc("TRN2", target_bir_lowering=False)
    x_d = nc.dram_tensor("xb", [T, C], fp16, kind="ExternalInput")
    m_d = nc.dram_tensor("M", [C, C], f32, kind="ExternalInput")
    out_d = nc.dram_tensor("out", [T, C], fp16, kind="ExternalOutput")

    with ExitStack() as ctx:
        tc = ctx.enter_context(tile.TileContext(nc))
        const = ctx.enter_context(tc.tile_pool(name="const", bufs=1))
        big = ctx.enter_context(tc.tile_pool(name="big", bufs=1))

        m_sb = const.tile([C, C], f32, tag="m")
        nc.sync.dma_start(out=m_sb, in_=m_d[:, :])
        m_r = const.tile([C, C], f32r, tag="m_r")
        nc.vector.tensor_copy(out=m_r, in_=m_sb)
        ident = const.tile([128, 128], f32, tag="ident")
        make_identity(nc, ident)
        maskneg = const.tile([128, 128], f32, tag="maskneg")
        make_causal_mask(nc, maskneg, mask_val=-1e9)

        gT = big.tile([C, T], f32r, tag="gT")
        x_hf = big.tile([128, NT, C], fp16, tag="x_hf")
        z_acc = big.tile([128, NT, C], fp16, tag="z_acc")

        # ---- setup: transpose x, g = x @ M ----
        nc.sync.dma_start(
            out=x_hf, in_=x_d[:, :].rearrange("(n p) c -> p n c", p=128)
        )
        with ExitStack() as sctx:
            xt_pool = sctx.enter_context(tc.tile_pool(name="xt_pool", bufs=1))
            setup_ps = sctx.enter_context(
                tc.tile_pool(name="setup_ps", bufs=2, space="PSUM")
            )
            x_sb = xt_pool.tile([128, NT, C], f32, tag="x_sb")
            nc.vector.tensor_copy(out=x_sb, in_=x_hf)
            xT = big.tile([C, T], f32r, tag="xT")
            for i in range(NT):
                ps_t = setup_ps.tile([C, 128], f32, tag="ps_t")
                nc.tensor.transpose(ps_t, x_sb[:, i, :], ident)
                nc.vector.tensor_copy(out=xT[:, i * 128 : (i + 1) * 128], in_=ps_t)
            for c8 in range(T // 512):
                sl = slice(c8 * 512, (c8 + 1) * 512)
                ps_g = setup_ps.tile([C, 512], f32, tag="ps_g")
                nc.tensor.matmul(
                    ps_g,
                    lhsT=m_r,
                    rhs=xT[:, sl],
                    start=True,
                    stop=True,
                )
                nc.vector.tensor_copy(out=gT[:, sl], in_=ps_g)

        # ---- flash loop over query tiles ----
        ps_s_pool = ctx.enter_context(tc.tile_pool(name="ps_s", bufs=3, space="PSUM"))
        ps_z_pool = ctx.enter_context(tc.tile_pool(name="ps_z", bufs=2, space="PSUM"))
        p_pool = ctx.enter_context(tc.tile_pool(name="p_pool", bufs=3))
        pt_pool = ctx.enter_context(tc.tile_pool(name="pt_pool", bufs=3))
        lil = ctx.enter_context(tc.tile_pool(name="lil", bufs=2))

        for i in range(NT):
            nk = i + 1  # causal: key tiles 0..i
            nchunks = (nk + 3) // 4
            ps_z = ps_z_pool.tile([128, C], f32, tag="ps_z")
            l_parts = lil.tile([128, 8], f32, tag="l_parts")
            for c in range(nchunks):
                k0 = c * 512
                ck = min(512, nk * 128 - k0)
                ntile = ck // 128
                ps_s = ps_s_pool.tile([128, 512], f32, tag="ps_s")
                nc.tensor.matmul(
                    ps_s[:, :ck],
                    lhsT=gT[:, i * 128 : (i + 1) * 128],
                    rhs=xT[:, k0 : k0 + ck],
                    start=True,
                    stop=True,
                )
                if c == nchunks - 1:
                    nc.vector.tensor_add(
                        out=ps_s[:, ck - 128 : ck],
                        in0=ps_s[:, ck - 128 : ck],
                        in1=maskneg,
                    )
                p_sb = p_pool.tile([128, 512], fp16, tag="p_sb")
                nc.scalar.activation(
                    out=p_sb[:, :ck],
                    in_=ps_s[:, :ck],
                    func=EXP,
                    scale=1.0,
                    accum_out=l_parts[:, c : c + 1],
                )
                pt = pt_pool.tile([128, 4, 128], fp16, tag="pt")
                nc.sync.dma_start(
                    out=pt[:, :ntile, :], in_=p_sb[:, :ck], transpose=True
                )
                for jj in range(ntile):
                    j = c * 4 + jj
                    nc.tensor.matmul(
                        ps_z,
                        lhsT=pt[:, jj, :],
                        rhs=x_hf[:, j, :],
                        start=(j == 0),
                        stop=(j == i),
                    )
            recip = lil.tile([128, 1], f32, tag="recip")
            if nchunks > 1:
                l_sum = lil.tile([128, 1], f32, tag="l_sum")
                nc.vector.reduce_sum(out=l_sum, in_=l_parts[:, :nchunks], axis=AXX)
                nc.vector.reciprocal(recip, l_sum)
            else:
                nc.vector.reciprocal(recip, l_parts[:, 0:1])
            nc.vector.tensor_scalar_mul(z_acc[:, i, :], ps_z, recip)

        nc.sync.dma_start(
            out=out_d[:, :].rearrange("(n p) c -> p n c", p=128), in_=z_acc
        )
    nc.finalize()
    return nc


def _get_nc():
    if "nc" not in _cache:
        _cache["nc"] = _build()
    return _cache["nc"]


def _get_callable():
    """Build the jitted per-core callables once; reuse across calls."""
    if "call" in _cache:
        return _cache["call"]

    import jax
    from jax.sharding import Mesh, PartitionSpec
    from jax.experimental.shard_map import shard_map
    import concourse.mybir as mybir
    from concourse.bass2jax import (
        _bass_exec_p,
        install_neuronx_cc_hook,
        partition_id_tensor,
    )

    install_neuronx_cc_hook()
    nc = _get_nc()
    partition_name = nc.partition_id_tensor.name if nc.partition_id_tensor else None

    in_names = []
    out_names = []
    out_avals = []
    for alloc in nc.m.functions[0].allocations:
        if not isinstance(alloc, mybir.MemoryLocationSet):
            continue
        name = alloc.memorylocations[0].name
        if alloc.kind == "ExternalInput":
            if name != partition_name:
                in_names.append(name)
        elif alloc.kind == "ExternalOutput":
            out_names.append(name)
            out_avals.append(
                jax.core.ShapedArray(tuple(alloc.tensor_shape), mybir.dt.np(alloc.dtype))
            )
    all_in_names = list(in_names)
    if partition_name is not None:
        all_in_names.append(partition_name)

    def _body(*args):
        operands = list(args)
        if partition_name is not None:
            operands.append(partition_id_tensor())
        outs = _bass_exec_p.bind(
            *operands,
            out_avals=tuple(out_avals),
            in_names=tuple(all_in_names),
            out_names=tuple(out_names),
            lowering_input_output_aliases=(),
            sim_require_finite=True,
            sim_require_nnan=True,
            nc=nc,
        )
        return tuple(outs)

    devices = jax.devices()[:N_CORES]
    assert len(devices) == N_CORES, f"need {N_CORES} devices, got {len(devices)}"
    calls = []
    for dev in devices:
        mesh = Mesh(np.asarray([dev]), ("core",))
        calls.append(
            jax.jit(
                shard_map(
                    _body,
                    mesh=mesh,
                    in_specs=(PartitionSpec("core"),) * len(in_names),
                    out_specs=(PartitionSpec("core"),) * len(out_names),
                    check_rep=False,
                ),
                keep_unused=True,
            )
        )
    pool = ThreadPoolExecutor(max_workers=N_CORES)
    _cache["call"] = (calls, in_names, pool)
    return _cache["call"]


def _host_prep(inputs):
    x = np.asarray(inputs["x"], dtype=np.float32)
    wq = np.asarray(inputs["Wq"], dtype=np.float32)
    wk = np.asarray(inputs["Wk"], dtype=np.float32)
    wv = np.asarray(inputs["Wv"], dtype=np.float32)
    x16 = x.astype(np.float16)
    m = np.ascontiguousarray((wq @ wk.T) * SCALE)  # [C, C] f32
    return x16, m, wv


def _run(inputs, trace=False):
    if trace:
        return _run_traced(inputs)
    calls, in_names, pool = _get_callable()
    x16, m, wv = _host_prep(inputs)
    arrs_w = {"M": m}
    out = np.empty((N_CORES, T, H), dtype=np.float32)

    def one(core):
        arrs = {"xb": np.ascontiguousarray(x16[core]), **arrs_w}
        try:
            o = calls[core](*[arrs[n] for n in in_names])
            z = np.asarray(o[0])
        except Exception:
            o = calls[core](*[arrs[n] for n in in_names])
            z = np.asarray(o[0])
        out[core] = z.astype(np.float32) @ wv  # epilogue: out = z @ Wv

    if "warm" not in _cache:
        # First call in this process: compile core 0's executable alone so
        # the NEFF lands in the on-disk cache before the other 7 compile.
        one(0)
        rest = [pool.submit(one, b) for b in range(1, N_CORES)]
        for f in rest:
            f.result()
        _cache["warm"] = True
    else:
        futs = [pool.submit(one, b) for b in range(N_CORES)]
        for f in futs:
            f.result()

    class _Res:
        exec_time_ns = None
        results = None

    return out, _Res()


def _run_traced(inputs):
    """Profiled path via run_bass_kernel_spmd (NTFF trace)."""
    from concourse.bass_utils import run_bass_kernel_spmd

    x16, m, wv = _host_prep(inputs)
    in_maps = [
        {"xb": np.ascontiguousarray(x16[b]), "M": m} for b in range(N_CORES)
    ]
    res = run_bass_kernel_spmd(
        _get_nc(), in_maps, core_ids=list(range(N_CORES)), trace=True
    )
    out = np.stack(
        [r["out"].astype(np.float32) @ wv for r in res.results], axis=0
    )
    return out, res


def kernel(x, Wq, Wk, Wv):
    out, _ = _run({"x": x, "Wq": Wq, "Wk": Wk, "Wv": Wv})
    return out
